# revision 6
# baseline (speedup 1.0000x reference)
"""Trainium2 Bass kernel: MultiHeadLatentAttention prefill (B=2, T=2048, D=2048,
H=16, HD=128, KVH=4, QL=1536, KVL=512).

Sharding: 8 cores = (batch b in {0,1}) x (kv-head group g in {0..3}).
Each core handles one batch element and the 4 q-heads of one kv head.
Host folds rms weights into up-projections, folds the (position = kv-head
index) K-rope rotation into Wkr, sums the 8 partial outputs at the end.

v3: all matmuls on float32r operands (measured ~3.3x faster per 512-free
matmul than bf16 on this target, near-f32 precision). Everything matmul-
facing is staged float32(r) in DRAM and moved with plain sync DMAs — v2's
gpsimd casting DMAs were the bottleneck (priced ~2.3x worse than HWDGE).
Phase 1 is a single pass with x fully resident f32r (128 KB/partition;
usable SBUF is ~243 KB). ql chunks spill to DRAM f32r and are re-streamed
in phase 2, which holds the Wuq/Wqr weights resident (48 KB) and computes
all 4 heads x {Q,Qr} per token tile in 8 PSUM banks. Softmax Z accumulates
on the PE via a ones-matmul per key block; the causal mask is an additive
0/-1e9 strip applied to PSUM scores before exp.

Device dataflow (feature-major activations [feat, T], f32r matmuls, f32 PSUM):
  ql/c/K chunks = W.T @ x ; rms stats via ones-matmul of Square(psum)
  kabs = Wuk.T @ c * 1/rms(c) ; V = c.T @ Wuv * 1/rms(c) per key block
  per token tile: 8 PSUM banks over 12 re-streamed ql chunks -> qh, rope(qr)
  per head, query tile j, key block s: S (2 matmuls, full width), +mask if
    diagonal, E = exp(S/16) f32r, Z += ones.T @ E, ctx += V_s.T @ E;
    ctx = pctx * recip(Z)
  out chunks = Wout.T @ ctx, written bf16; host sums over g.
"""

import numpy as np
import ml_dtypes

B, T, D = 2, 2048, 2048
H, HD, KVH = 16, 128, 4
QL, KVL = 1536, 512
G = KVH                  # core groups per batch
HPG = H // KVH           # q heads per group
NCORES = B * G
TS = 512                 # free-dim tile
NT = T // TS             # 4
DCH = D // 128           # 16
QLCH = QL // 128         # 12
CCH = KVL // 128         # 4
SCH = T // 128           # 16
EPS = 1e-6
SM_SCALE = 1.0 / 16.0    # 1/sqrt(2*HD)
BF16 = ml_dtypes.bfloat16

_CACHE = {}
LAST_RESULTS = None


def _build_program(reps=1):
    import concourse.bacc as bacc
    import concourse.tile as tile
    from concourse import mybir
    from concourse.bass import ts

    bf = mybir.dt.bfloat16
    f32 = mybir.dt.float32
    f32r = mybir.dt.float32r
    AF = mybir.ActivationFunctionType
    SWAP_MASK = [i ^ 1 for i in range(32)]

    nc = bacc.Bacc("TRN2", target_bir_lowering=False, debug=False)

    xT = nc.dram_tensor("x_T", [D, T], f32r, kind="ExternalInput")
    wdqT = nc.dram_tensor("wdqT", [D, QL], f32r, kind="ExternalInput")
    wdkvT = nc.dram_tensor("wdkvT", [D, KVL], f32r, kind="ExternalInput")
    wkrT = nc.dram_tensor("wkrT", [D, HD], f32r, kind="ExternalInput")
    # [12 ql-chunks][128 ql][8 = (h,proj) pairs][128 hd]
    wuqr = nc.dram_tensor("wuqr", [QLCH, 128, 2 * HPG, HD], f32r,
                          kind="ExternalInput")
    wukT = nc.dram_tensor("wukT", [KVL, HD], f32r, kind="ExternalInput")
    wuvT = nc.dram_tensor("wuvT", [KVL, HD], f32r, kind="ExternalInput")
    # [16 d-chunks][128 = c-dim within head][4 heads][128 d-rows]
    woutE = nc.dram_tensor("woutE", [DCH, 128, HPG, 128], f32r,
                           kind="ExternalInput")
    ctab = nc.dram_tensor("ctab", [HD, T], f32, kind="ExternalInput")
    stab = nc.dram_tensor("stab", [HD, T], f32, kind="ExternalInput")
    maskA = nc.dram_tensor("maskA", [128, 896], f32, kind="ExternalInput")
    onesR = nc.dram_tensor("onesR", [128, 128], f32r, kind="ExternalInput")
    outT = nc.dram_tensor("out_T", [D, T], bf, kind="ExternalOutput")

    # phase-1 chunk schedule: c chunks first, then K, then ql chunks
    M_TOTAL = CCH + 1 + QLCH

    def proj_src(m):
        if m < CCH:
            return wdkvT.ap()[:, m * 128:(m + 1) * 128]
        if m == CCH:
            return wkrT.ap()
        return wdqT.ap()[:, (m - CCH - 1) * 128:(m - CCH) * 128]

    with tile.TileContext(nc) as tc:
      for _rep in range(reps):
        with (
            tc.tile_pool(name="A", bufs=1) as A,
            tc.tile_pool(name="DRS", bufs=1, space="DRAM") as DRS,
        ):
            kk_sb = A.tile([128, SCH, 2, HD], f32r)   # [s][kabs|k]
            v_sb = A.tile([128, SCH, HD], f32r)
            wuk_sb = A.tile([128, CCH, HD], f32r)
            wuv_sb = A.tile([128, CCH, HD], f32r)
            ones_sb = A.tile([128, 128], f32r)
            sq_row = A.tile([1, T], f32)
            sc_row = A.tile([1, T], f32)
            sc_col = A.tile([128, SCH], f32)
            eps_sb = A.tile([1, 1], f32)
            sqq_bc = A.tile([128, T], f32)
            scc_bc = A.tile([128, T], f32)
            ql_dram = DRS.tile([QLCH, 128, T], f32r)
            sc_dram = DRS.tile([1, T], f32)

            nc.sync.dma_start(out=ones_sb, in_=onesR.ap())
            nc.vector.memset(eps_sb, EPS)

            # ---------------- phase 1: latent projections from x -------------
            with (
                tc.tile_pool(name="P1", bufs=1) as P1,
                tc.tile_pool(name="P1S", bufs=2) as P1S,
                tc.tile_pool(name="SQP", bufs=2) as SQP,
                tc.tile_pool(name="QSP", bufs=2) as QSP,
                tc.tile_pool(name="PP1", bufs=3, space="PSUM") as PP1,
                tc.tile_pool(name="PZ1", bufs=4, space="PSUM") as PZ1,
            ):
                nc.sync.dma_start(
                    out=wuk_sb,
                    in_=wukT.ap().rearrange("(c p) d -> p c d", p=128),
                )
                nc.sync.dma_start(
                    out=wuv_sb,
                    in_=wuvT.ap().rearrange("(c p) d -> p c d", p=128),
                )
                xT_r = xT.ap().rearrange("(c p) t -> c p t", p=128)
                TH = T // 2
                for hp in range(2):
                    PC = slice(hp * TH, (hp + 1) * TH)   # pass columns
                    x_sb = P1.tile([128, DCH, TH], f32r, tag="xh", name="xh")
                    for d in range(DCH):
                        nc.sync.dma_start(out=x_sb[:, d, :], in_=xT_r[d][:, PC])
                    c_sb = P1.tile([128, CCH, TH], f32r, tag="ch", name="ch")

                    psz_c = [None] * 2
                    psz_q = [None] * 2
                    for m in range(M_TOTAL):
                        w_t = P1S.tile([128, DCH, 128], f32r, tag="wstream")
                        nc.sync.dma_start(
                            out=w_t,
                            in_=proj_src(m).rearrange("(c p) f -> p c f", p=128),
                        )
                        is_c = m < CCH
                        is_k = m == CCH
                        for t in range(2):
                            TC = slice(hp * TH + t * TS, hp * TH + (t + 1) * TS)
                            ps = PP1.tile([128, TS], f32, tag="pp")
                            for d in range(DCH):
                                nc.tensor.matmul(
                                    ps,
                                    lhsT=w_t[:, d, :],
                                    rhs=x_sb[:, d, ts(t, TS)],
                                    start=(d == 0),
                                    stop=(d == DCH - 1),
                                )
                            tg = 2 * hp + t   # global tile index
                            if is_k:
                                nc.vector.tensor_copy(
                                    kk_sb[:, 4 * tg:4 * tg + 4, 1, :], ps
                                )
                                continue
                            sq = SQP.tile([128, TS], f32r, tag="sq")
                            nc.scalar.activation(sq, ps, AF.Square)
                            psz = psz_c if is_c else psz_q
                            mi = m if is_c else m - CCH - 1
                            nch = CCH if is_c else QLCH
                            if mi == 0:
                                psz[t] = PZ1.tile([1, TS], f32, tag="pz",
                                                  name="psz")
                            nc.tensor.matmul(
                                psz[t],
                                lhsT=ones_sb[:, 0:1],
                                rhs=sq,
                                start=(mi == 0),
                                stop=(mi == nch - 1),
                            )
                            if is_c:
                                nc.vector.tensor_copy(
                                    c_sb[:, m, ts(t, TS)], ps
                                )
                            else:
                                qsp = QSP.tile([128, TS], f32r, tag="qspill")
                                nc.vector.tensor_copy(qsp, ps)
                                nc.sync.dma_start(
                                    out=ql_dram[mi, :, TC], in_=qsp
                                )
                            if mi == nch - 1:
                                row = sc_row if is_c else sq_row
                                nrm = KVL if is_c else QL
                                nc.scalar.activation(
                                    row[0:1, TC],
                                    psz[t],
                                    AF.Sqrt,
                                    bias=eps_sb[0:1, 0:1],
                                    scale=1.0 / nrm,
                                )
                        if m == CCH - 1:
                            nc.vector.reciprocal(
                                sc_row[0:1, PC], sc_row[0:1, PC]
                            )
                            nc.gpsimd.partition_broadcast(
                                scc_bc[:, PC], sc_row[0:1, PC]
                            )
                            nc.sync.dma_start(
                                out=sc_dram[:, PC], in_=sc_row[0:1, PC]
                            )
                            nc.sync.dma_start(
                                out=sc_col[:, 8 * hp:8 * hp + 8],
                                in_=sc_dram[:, PC].rearrange(
                                    "o (s p) -> (o p) s", p=128
                                ),
                            )
                        if m == M_TOTAL - 1:
                            nc.vector.reciprocal(
                                sq_row[0:1, PC], sq_row[0:1, PC]
                            )
                            nc.gpsimd.partition_broadcast(
                                sqq_bc[:, PC], sq_row[0:1, PC]
                            )

                    # kabs + V epilogues for this pass
                    for t in range(2):
                        tg = 2 * hp + t
                        TC = slice(hp * TH + t * TS, hp * TH + (t + 1) * TS)
                        ps = PP1.tile([128, TS], f32, tag="pp")
                        for cc in range(CCH):
                            nc.tensor.matmul(
                                ps,
                                lhsT=wuk_sb[:, cc, :],
                                rhs=c_sb[:, cc, ts(t, TS)],
                                start=(cc == 0),
                                stop=(cc == CCH - 1),
                            )
                        nc.vector.tensor_mul(
                            kk_sb[:, 4 * tg:4 * tg + 4, 0, :], ps,
                            scc_bc[:, TC],
                        )
                    for sl in range(8):
                        s = 8 * hp + sl
                        ps = PP1.tile([128, TS], f32, tag="pp")
                        for cc in range(CCH):
                            nc.tensor.matmul(
                                ps[:, 0:HD],
                                lhsT=c_sb[:, cc, sl * 128:(sl + 1) * 128],
                                rhs=wuv_sb[:, cc, :],
                                start=(cc == 0),
                                stop=(cc == CCH - 1),
                            )
                        nc.vector.tensor_scalar_mul(
                            v_sb[:, s, :], ps[:, 0:HD], sc_col[:, s:s + 1]
                        )

            # ---------------- phases 2-4 under the qq pool -------------------
            with tc.tile_pool(name="QQ", bufs=1) as QQ:
              qq_sb = QQ.tile([128, HPG, 2, T], f32r)   # [h][q|qr][t]
              # ---------------- phase 2: all-heads Q/Qr ----------------------
              with (
                  tc.tile_pool(name="P2T", bufs=1) as P2T,
                  tc.tile_pool(name="P2S", bufs=3) as P2S,
                  tc.tile_pool(name="TMP2", bufs=4) as TMP2,
                  tc.tile_pool(name="PQ", bufs=1, space="PSUM") as PQ,
              ):
                ctab_sb = P2T.tile([128, T], f32)
                stab_sb = P2T.tile([128, T], f32)
                wqr_sb = P2T.tile([128, QLCH, 2 * HPG, HD], f32r)
                nc.sync.dma_start(out=ctab_sb, in_=ctab.ap())
                nc.sync.dma_start(out=stab_sb, in_=stab.ap())
                nc.sync.dma_start(
                    out=wqr_sb,
                    in_=wuqr.ap().rearrange("c p i d -> p c i d"),
                )
                for t in range(NT):
                    banks = [
                        PQ.tile([128, TS], f32, tag=f"pq{i}", name=f"pq{i}")
                        for i in range(2 * HPG)
                    ]
                    for m in range(QLCH):
                        ql_t = P2S.tile([128, TS], f32r, tag="qlin")
                        nc.sync.dma_start(
                            out=ql_t, in_=ql_dram[m, :, ts(t, TS)]
                        )
                        for i in range(2 * HPG):
                            nc.tensor.matmul(
                                banks[i],
                                lhsT=wqr_sb[:, m, i, :],
                                rhs=ql_t,
                                start=(m == 0),
                                stop=(m == QLCH - 1),
                            )
                    for h in range(HPG):
                        nc.vector.tensor_mul(
                            qq_sb[:, h, 0, ts(t, TS)],
                            banks[2 * h], sqq_bc[:, ts(t, TS)],
                        )
                        psr = banks[2 * h + 1]
                        tsw = TMP2.tile([128, TS], f32, tag="tmp")
                        nc.vector.stream_shuffle(tsw, psr, SWAP_MASK)
                        t1 = TMP2.tile([128, TS], f32, tag="tmp")
                        nc.vector.tensor_mul(t1, psr, ctab_sb[:, ts(t, TS)])
                        t2 = TMP2.tile([128, TS], f32, tag="tmp")
                        nc.vector.tensor_mul(t2, tsw, stab_sb[:, ts(t, TS)])
                        t3 = TMP2.tile([128, TS], f32, tag="tmp")
                        nc.vector.tensor_add(t3, t1, t2)
                        nc.vector.tensor_mul(
                            qq_sb[:, h, 1, ts(t, TS)],
                            t3, sqq_bc[:, ts(t, TS)],
                        )

              # ---------------- phase 3: attention ---------------------------
              with tc.tile_pool(name="CTX", bufs=1) as CTX:
                ctx_sb = [
                    [
                        CTX.tile([128, TS], f32r, tag=f"ctx{h}_{j}",
                                 name=f"ctx{h}_{j}")
                        for j in range(NT)
                    ]
                    for h in range(HPG)
                ]
                masks_sb = CTX.tile([128, 896], f32)
                nc.sync.dma_start(out=masks_sb, in_=maskA.ap())
                with (
                    tc.tile_pool(name="EP", bufs=6) as EP,
                    tc.tile_pool(name="TMP3", bufs=3) as TMP3,
                    tc.tile_pool(name="PB", bufs=3, space="PSUM") as PB,
                    tc.tile_pool(name="PCT", bufs=2, space="PSUM") as PCT,
                    tc.tile_pool(name="PZB", bufs=2, space="PSUM") as PZB,
                ):
                    for h in range(HPG):
                        for j in range(NT):
                            n_s = 4 * (j + 1)
                            pctx = PCT.tile([128, TS], f32, tag="pct")
                            zps = PZB.tile([128, TS], f32, tag="pzb")
                            for s in range(n_s):
                                r = s - 4 * j
                                pss = PB.tile([128, TS], f32, tag="pb")
                                nc.tensor.matmul(
                                    pss,
                                    lhsT=kk_sb[:, s, 0, :],
                                    rhs=qq_sb[:, h, 0, ts(j, TS)],
                                    start=True,
                                    stop=False,
                                )
                                nc.tensor.matmul(
                                    pss,
                                    lhsT=kk_sb[:, s, 1, :],
                                    rhs=qq_sb[:, h, 1, ts(j, TS)],
                                    start=False,
                                    stop=True,
                                )
                                if r >= 0:
                                    nc.vector.tensor_add(
                                        pss, pss,
                                        masks_sb[:, 384 - 128 * r:
                                                 896 - 128 * r],
                                    )
                                e_t = EP.tile([128, TS], f32r, tag="e")
                                nc.scalar.activation(
                                    e_t, pss, AF.Exp, scale=SM_SCALE
                                )
                                nc.tensor.matmul(
                                    zps,
                                    lhsT=ones_sb,
                                    rhs=e_t,
                                    start=(s == 0),
                                    stop=(s == n_s - 1),
                                )
                                nc.tensor.matmul(
                                    pctx,
                                    lhsT=v_sb[:, s, :],
                                    rhs=e_t,
                                    start=(s == 0),
                                    stop=(s == n_s - 1),
                                )
                            zrec = TMP3.tile([128, TS], f32, tag="zr")
                            nc.vector.reciprocal(zrec, zps)
                            nc.vector.tensor_mul(ctx_sb[h][j], pctx, zrec)

                # ---------------- phase 4: output projection -----------------
                with (
                    tc.tile_pool(name="P4W", bufs=3) as P4W,
                    tc.tile_pool(name="P4O", bufs=2) as P4O,
                    tc.tile_pool(name="PP4", bufs=3, space="PSUM") as PP4,
                ):
                    for e in range(DCH):
                        wo_t = P4W.tile([128, HPG, 128], f32r, tag="wo")
                        nc.sync.dma_start(out=wo_t, in_=woutE.ap()[e])
                        o_t = P4O.tile([128, T], bf, tag="ot")
                        for t in range(NT):
                            ps = PP4.tile([128, TS], f32, tag="pp4")
                            for q in range(HPG):
                                nc.tensor.matmul(
                                    ps,
                                    lhsT=wo_t[:, q, :],
                                    rhs=ctx_sb[q][t],
                                    start=(q == 0),
                                    stop=(q == HPG - 1),
                                )
                            nc.vector.tensor_copy(o_t[:, ts(t, TS)], ps)
                        nc.sync.dma_start(
                            out=outT.ap()[e * 128:(e + 1) * 128, :], in_=o_t
                        )

    nc.compile()
    return nc


def _get_program():
    if "nc" not in _CACHE:
        _CACHE["nc"] = _build_program()
    return _CACHE["nc"]


def _host_prep(inputs):
    """Fold weights on the host and build the 8 per-core input maps."""
    x = np.asarray(inputs["x"], np.float32)
    Wdq = np.asarray(inputs["Wdq"], np.float32)
    qw = np.asarray(inputs["q_norm_w"], np.float32)
    Wuq = np.asarray(inputs["Wuq"], np.float32) * qw[None, :]
    Wqr = np.asarray(inputs["Wqr"], np.float32) * qw[None, :]
    Wdkv = np.asarray(inputs["Wdkv"], np.float32)
    kvw = np.asarray(inputs["kv_norm_w"], np.float32)
    Wuk = np.asarray(inputs["Wuk"], np.float32) * kvw[None, :]
    Wuv = np.asarray(inputs["Wuv"], np.float32) * kvw[None, :]
    Wkr = np.asarray(inputs["Wkr"], np.float32)
    Wout = np.asarray(inputs["Wout"], np.float32)

    inv = 1.0 / (10000.0 ** (np.arange(0, HD, 2, dtype=np.float32) / HD))
    f = np.arange(T, dtype=np.float32)[None, :] * inv[:, None]   # [64, T]
    cosT, sinT = np.cos(f), np.sin(f)
    Ctab = np.repeat(cosT, 2, axis=0).astype(np.float32)         # [128, T]
    Stab = np.repeat(sinT, 2, axis=0).astype(np.float32)
    Stab[0::2, :] *= -1.0                                        # pair-swap sign

    fH = np.arange(KVH, dtype=np.float32)[None, :] * inv[:, None]  # [64, KVH]
    cosH, sinH = np.cos(fH), np.sin(fH)

    # additive causal strip: 0 where allowed (u >= p + 384), else -1e9
    p_idx = np.arange(128)[:, None]
    u_idx = np.arange(896)[None, :]
    maskA = np.where(u_idx >= p_idx + 384, 0.0, -1e9).astype(np.float32)
    onesR = np.ones((128, 128), np.float32)

    def f32c(a):
        return np.ascontiguousarray(a, dtype=np.float32)

    wdqT = f32c(Wdq.T)
    wdkvT = f32c(Wdkv.T)
    wukT = f32c(Wuk.T)

    in_maps = []
    for b in range(B):
        x_T = f32c(x[b].T)
        for g in range(G):
            # fold K-rope (fixed rotation per kv-head index) into Wkr
            Wkr_g = Wkr[g * HD:(g + 1) * HD, :]
            we, wo = Wkr_g[0::2, :], Wkr_g[1::2, :]
            c_g, s_g = cosH[:, g][:, None], sinH[:, g][:, None]
            Wkr_eff = np.empty_like(Wkr_g)
            Wkr_eff[0::2, :] = we * c_g - wo * s_g
            Wkr_eff[1::2, :] = we * s_g + wo * c_g

            # wuqr: [12 ql-chunks][128 ql][8 = (h,proj)][128 hd]
            Wuq_g = Wuq[g * HPG * HD:(g + 1) * HPG * HD].T   # [QL, 512]
            Wqr_g = Wqr[g * HPG * HD:(g + 1) * HPG * HD].T
            wq = np.empty((QL, 2 * HPG, HD), np.float32)
            for h in range(HPG):
                wq[:, 2 * h, :] = Wuq_g[:, h * HD:(h + 1) * HD]
                wq[:, 2 * h + 1, :] = Wqr_g[:, h * HD:(h + 1) * HD]
            wuqr_g = wq.reshape(QLCH, 128, 2 * HPG, HD)

            # woutE: [16 d-chunks][128 c-of-head][4 heads][128 d-rows]
            Wout_g = Wout[:, g * HPG * HD:(g + 1) * HPG * HD]  # [D, 512]
            woutE = np.empty((DCH, 128, HPG, 128), np.float32)
            for e in range(DCH):
                blk = Wout_g[e * 128:(e + 1) * 128]            # [128 d, 512]
                for q in range(HPG):
                    woutE[e, :, q, :] = blk[:, q * 128:(q + 1) * 128].T

            in_maps.append(
                dict(
                    x_T=x_T,
                    wdqT=wdqT,
                    wdkvT=wdkvT,
                    wkrT=f32c(Wkr_eff.T),
                    wuqr=f32c(wuqr_g),
                    wukT=wukT,
                    wuvT=f32c(Wuv[g * HD:(g + 1) * HD].T),
                    woutE=f32c(woutE),
                    ctab=Ctab,
                    stab=Stab,
                    maskA=maskA,
                    onesR=onesR,
                )
            )
    return in_maps


def kernel(**inputs):
    global LAST_RESULTS
    from concourse import bass_utils

    nc = _get_program()
    in_maps = _host_prep(inputs)
    res = bass_utils.run_bass_kernel_spmd(
        nc, in_maps, core_ids=list(range(NCORES))
    )
    LAST_RESULTS = res
    out = np.zeros((B, T, D), np.float32)
    for i, r in enumerate(res.results):
        out[i // G] += r["out_T"].T.astype(np.float32)
    return out


# revision 7
# speedup vs baseline: 1.7912x; 1.7912x over previous
"""Trainium2 Bass kernel: MultiHeadLatentAttention prefill (B=2, T=2048, D=2048,
H=16, HD=128, KVH=4, QL=1536, KVL=512).

Sharding: 8 cores = (batch b in {0,1}) x (kv-head group g in {0..3}).
Each core handles one batch element and the 4 q-heads of one kv head.
Host folds rms weights into up-projections, folds the (position = kv-head
index) K-rope rotation into Wkr, sums the 8 partial outputs at the end.

Changes vs the original baseline: softmax Z via per-block DVE accumulation
of E into eacc plus ONE [128x128]-ones matmul per (head, tile) whose output
is already broadcast across partitions (the 160 per-block M=1 ones-matmuls
and all per-head gpsimd broadcasts are gone - those cost ~0.5ms on HW),
causal masks sliced from a single sheared strip, bf16 partial output,
deeper attention PSUM pools (PB=4) for score pipelining.

Device dataflow (feature-major activations [feat, T], bf16 matmuls, fp32 PSUM):
  ql_T = WdqT.T @ x_T ; c_T = WdkvT.T @ x_T ; K_T = WkrT.T @ x_T (rope folded)
  rms scales for ql/c via ones-matmul column sums of squares, applied at
  consumer epilogues:
    Q_T  = (WuqT.T @ ql_T) * sq[t]
    Qr_T = rope(WqrT.T @ ql_T) * sq[t]   (rope via stream_shuffle pair-swap)
    Kabs_T = (WukT.T @ c_T) * sc[t]      (absorbed-Wuk trick, shared by heads)
    V    = (c_T slices).T @ WuvT * sc[s] (token-major; per-partition scale)
  per head, query tile j (1024 wide): for key block s (128 keys):
    S_T[s-part, q] = Kabs_T[:,s].T @ Q_T + K_T[:,s].T @ Qr_T
    E = exp(S_T/16); diagonal blocks masked via sheared-strip multiply
    eacc += E (DVE);  ctx_psum += V_s.T @ E
  Z[all parts] = ones128.T @ eacc;  ctx = ctx_psum * reciprocal(Z)
  out_T = WoutT.T @ ctx_T  (partial over heads; host sums over g)
"""

import numpy as np
import ml_dtypes

B, T, D = 2, 2048, 2048
H, HD, KVH = 16, 128, 4
QL, KVL = 1536, 512
G = KVH                  # core groups per batch
HPG = H // KVH           # q heads per group
NCORES = B * G
TS = 512                 # free-dim tile
NT = T // TS             # 4
DCH = D // 128           # 16
QLCH = QL // 128         # 12
CCH = KVL // 128         # 4
SCH = T // 128           # 16
EPS = 1e-6
SM_SCALE = 1.0 / 16.0    # 1/sqrt(2*HD)
BF16 = ml_dtypes.bfloat16

_CACHE = {}
LAST_RESULTS = None


def _build_program(reps=1):
    import concourse.bacc as bacc
    import concourse.tile as tile
    from concourse import mybir
    from concourse.bass import ts

    bf = mybir.dt.bfloat16
    f32 = mybir.dt.float32
    AF = mybir.ActivationFunctionType
    SWAP_MASK = [i ^ 1 for i in range(32)]

    nc = bacc.Bacc("TRN2", target_bir_lowering=False, debug=False)

    xT = nc.dram_tensor("x_T", [D, T], bf, kind="ExternalInput")
    wdqT = nc.dram_tensor("wdqT", [D, QL], bf, kind="ExternalInput")
    wdkvT = nc.dram_tensor("wdkvT", [D, KVL], bf, kind="ExternalInput")
    wkrT = nc.dram_tensor("wkrT", [D, HD], bf, kind="ExternalInput")
    wuqT = nc.dram_tensor("wuqT", [QL, HPG * HD], bf, kind="ExternalInput")
    wqrT = nc.dram_tensor("wqrT", [QL, HPG * HD], bf, kind="ExternalInput")
    wukT = nc.dram_tensor("wukT", [KVL, HD], bf, kind="ExternalInput")
    wuvT = nc.dram_tensor("wuvT", [KVL, HD], bf, kind="ExternalInput")
    woutT = nc.dram_tensor("woutT", [HPG * HD, D], bf, kind="ExternalInput")
    ctab = nc.dram_tensor("ctab", [HD, T], bf, kind="ExternalInput")
    stab = nc.dram_tensor("stab", [HD, T], bf, kind="ExternalInput")
    outT = nc.dram_tensor("out_T", [D, T], bf, kind="ExternalOutput")

    # phase-1 projection schedule: c chunks first (so the rms-scale chain for
    # the kv side completes early), then K, then ql chunks
    M_TOTAL = CCH + 1 + QLCH

    def proj_src(m):
        if m < CCH:
            return wdkvT.ap()[:, m * 128:(m + 1) * 128]
        if m == CCH:
            return wkrT.ap()
        return wdqT.ap()[:, (m - CCH - 1) * 128:(m - CCH) * 128]

    with tile.TileContext(nc) as tc:
      for _rep in range(reps):
        with (
            tc.tile_pool(name="A", bufs=1) as A,
            tc.tile_pool(name="QLP", bufs=1) as QLP,
        ):
            c_sb = A.tile([128, CCH, T], bf)
            k_sb = A.tile([128, T], bf)
            kabs_sb = A.tile([128, T], bf)
            v_sb = A.tile([128, SCH, HD], bf)
            wuk_sb = A.tile([128, CCH, HD], bf)
            wuv_sb = A.tile([128, CCH, HD], bf)
            ones_mat = A.tile([128, 128], bf)
            ones_sb = A.tile([128, 1], bf)
            sq_row = A.tile([1, T], f32)
            sc_row = A.tile([1, T], f32)
            sc_col = A.tile([128, SCH], f32)   # column form of sc (for V)
            eps_sb = A.tile([1, 1], f32)
            ql_sb = QLP.tile([128, QLCH, T], bf)
            sqq_bc = QLP.tile([128, T], f32)   # broadcast of 1/rms(ql)

            # ---------------- phase 1: latent projections from x -------------
            with (
                tc.tile_pool(name="PH1", bufs=1) as P1,
                tc.tile_pool(name="P1S", bufs=3) as P1S,
                tc.tile_pool(name="DRS", bufs=1, space="DRAM") as DRS,
                tc.tile_pool(name="PP1", bufs=3, space="PSUM") as PP1,
                tc.tile_pool(name="PZ1", bufs=4, space="PSUM") as PZ1,
            ):
                scc_bc = P1.tile([128, T], f32)    # broadcast of 1/rms(c)
                xT_r = xT.ap().rearrange("(c p) t -> c p t", p=128)
                x_sb = []
                for d in range(DCH):
                    xd = P1.tile([128, T], bf, tag=f"x{d}", name=f"x{d}")
                    nc.gpsimd.dma_start(out=xd, in_=xT_r[d])
                    x_sb.append(xd)

                w_ts = []
                for m in range(M_TOTAL):
                    w_t = P1S.tile([128, DCH, 128], bf, tag="wstream")
                    nc.sync.dma_start(
                        out=w_t, in_=proj_src(m).rearrange("(c p) f -> p c f", p=128)
                    )
                    w_ts.append(w_t)
                    if m == 1:
                        # small constants after the first two weight slices
                        nc.vector.memset(ones_mat, 1.0)
                        nc.vector.memset(ones_sb, 1.0)
                        nc.vector.memset(eps_sb, EPS)
                        nc.sync.dma_start(
                            out=wuk_sb,
                            in_=wukT.ap().rearrange("(c p) d -> p c d", p=128),
                        )
                        nc.sync.dma_start(
                            out=wuv_sb,
                            in_=wuvT.ap().rearrange("(c p) d -> p c d", p=128),
                        )

                # psz[t] accumulate sum-of-squares across chunks (c then ql)
                psz_c = [None] * NT
                psz_q = [None] * NT
                for m in range(M_TOTAL):
                    w_t = w_ts[m]
                    is_c = m < CCH
                    is_k = m == CCH
                    for t in range(NT):
                        ps = PP1.tile([128, TS], f32, tag="pp")
                        for d in range(DCH):
                            nc.tensor.matmul(
                                ps,
                                lhsT=w_t[:, d, :],
                                rhs=x_sb[d][:, ts(t, TS)],
                                start=(d == 0),
                                stop=(d == DCH - 1),
                            )
                        if is_c:
                            dst = c_sb[:, m, ts(t, TS)]
                        elif is_k:
                            dst = k_sb[:, ts(t, TS)]
                        else:
                            dst = ql_sb[:, m - CCH - 1, ts(t, TS)]
                        nc.vector.tensor_copy(dst, ps)
                        if is_k:
                            continue
                        # interleaved rms stats on the bf16 copy
                        sq = P1S.tile([128, TS], bf, tag="wstream")
                        nc.vector.tensor_mul(sq, dst, dst)
                        psz = psz_c if is_c else psz_q
                        mi = m if is_c else m - CCH - 1
                        nch = CCH if is_c else QLCH
                        if mi == 0:
                            psz[t] = PZ1.tile([1, TS], f32, tag="pz", name="psz")
                        nc.tensor.matmul(
                            psz[t],
                            lhsT=ones_sb,
                            rhs=sq,
                            start=(mi == 0),
                            stop=(mi == nch - 1),
                        )
                        if mi == nch - 1:
                            row = sc_row if is_c else sq_row
                            nrm = KVL if is_c else QL
                            nc.scalar.activation(
                                row[0:1, ts(t, TS)],
                                psz[t],
                                AF.Sqrt,
                                bias=eps_sb[0:1, 0:1],
                                scale=1.0 / nrm,
                            )
                    # scale chains as soon as each row completes
                    if m == CCH - 1:
                        nc.vector.reciprocal(sc_row[0:1, :], sc_row[0:1, :])
                        nc.gpsimd.partition_broadcast(scc_bc, sc_row[0:1, :])
                        # column form of sc via DRAM round-trip
                        dr = DRS.tile([1, T], f32)
                        nc.sync.dma_start(out=dr, in_=sc_row[0:1, :])
                        nc.sync.dma_start(
                            out=sc_col,
                            in_=dr[:, :].rearrange("o (s p) -> (o p) s", p=128),
                        )
                    if m == M_TOTAL - 1:
                        nc.vector.reciprocal(sq_row[0:1, :], sq_row[0:1, :])
                        nc.gpsimd.partition_broadcast(sqq_bc, sq_row[0:1, :])

                # Kabs (absorbed Wuk) and token-major V from raw c + epilogue
                for t in range(NT):
                    ps = PP1.tile([128, TS], f32, tag="pp")
                    for cc in range(CCH):
                        nc.tensor.matmul(
                            ps,
                            lhsT=wuk_sb[:, cc, :],
                            rhs=c_sb[:, cc, ts(t, TS)],
                            start=(cc == 0),
                            stop=(cc == CCH - 1),
                        )
                    nc.vector.tensor_mul(
                        kabs_sb[:, ts(t, TS)], ps, scc_bc[:, ts(t, TS)]
                    )
                for s in range(SCH):
                    ps = PP1.tile([128, TS], f32, tag="pp")
                    for cc in range(CCH):
                        nc.tensor.matmul(
                            ps[:, 0:HD],
                            lhsT=c_sb[:, cc, s * 128:(s + 1) * 128],
                            rhs=wuv_sb[:, cc, :],
                            start=(cc == 0),
                            stop=(cc == CCH - 1),
                        )
                    nc.vector.tensor_scalar_mul(
                        v_sb[:, s, :], ps[:, 0:HD], sc_col[:, s:s + 1]
                    )

            # ---------------- phases 2+3: per-head Q/Qr + attention ----------
            with tc.tile_pool(name="P3B", bufs=1) as P3B:
                # per-(head, j) ctx tiles so phase 4 can start on early tiles
                ctx_sb = [
                    [
                        P3B.tile([128, TS], bf, tag=f"ctx{h}_{j}", name=f"ctx{h}_{j}")
                        for j in range(NT)
                    ]
                    for h in range(HPG)
                ]
                wout_sb = P3B.tile([128, HPG, T], bf)
                ctab_sb = P3B.tile([128, T], bf)
                stab_sb = P3B.tile([128, T], bf)
                # single sheared causal strip: strip[p, u] = 1 iff u >= p + 384;
                # the mask for diagonal offset r is strip[:, 128*(3-r):][0:TS]
                masks_sb = P3B.tile([128, 896], bf)
                nc.vector.memset(masks_sb, 1.0)
                nc.gpsimd.affine_select(
                    out=masks_sb,
                    in_=masks_sb,
                    pattern=[[1, 896]],
                    compare_op=mybir.AluOpType.is_ge,
                    fill=0.0,
                    base=-(128 * 3),
                    channel_multiplier=-1,
                )
                with (
                    tc.tile_pool(name="P3S", bufs=2) as P3S,
                    tc.tile_pool(name="EP", bufs=10) as EP,
                    tc.tile_pool(name="TMPP", bufs=4) as TMPP,
                    tc.tile_pool(name="ZR", bufs=2) as ZR,
                    tc.tile_pool(name="PB", bufs=3, space="PSUM") as PB,
                    tc.tile_pool(name="PCT", bufs=3, space="PSUM") as PCT,
                    tc.tile_pool(name="PZB", bufs=2, space="PSUM") as PZB,
                ):
                    for h in range(HPG):
                        wuq_t = P3S.tile([128, QLCH, HD], bf, tag="wuq")
                        wqr_t = P3S.tile([128, QLCH, HD], bf, tag="wqr")
                        nc.sync.dma_start(
                            out=wuq_t,
                            in_=wuqT.ap()[:, h * HD:(h + 1) * HD].rearrange(
                                "(c p) f -> p c f", p=128
                            ),
                        )
                        nc.sync.dma_start(
                            out=wqr_t,
                            in_=wqrT.ap()[:, h * HD:(h + 1) * HD].rearrange(
                                "(c p) f -> p c f", p=128
                            ),
                        )
                        if h == 0:
                            # prefetch phase-3/4 constants behind head-0 weights
                            nc.sync.dma_start(out=ctab_sb, in_=ctab.ap())
                            nc.sync.dma_start(out=stab_sb, in_=stab.ap())
                            nc.gpsimd.dma_start(
                                out=wout_sb,
                                in_=woutT.ap().rearrange("(c p) e -> p c e", p=128),
                            )
                        qh_sb = P3S.tile([128, T], bf, tag="qh")
                        qrh_sb = P3S.tile([128, T], bf, tag="qrh")
                        for t in range(NT):
                            psq = PB.tile([128, TS], f32, tag="pb")
                            for m in range(QLCH):
                                nc.tensor.matmul(
                                    psq,
                                    lhsT=wuq_t[:, m, :],
                                    rhs=ql_sb[:, m, ts(t, TS)],
                                    start=(m == 0),
                                    stop=(m == QLCH - 1),
                                )
                            nc.vector.tensor_mul(
                                qh_sb[:, ts(t, TS)], psq, sqq_bc[:, ts(t, TS)]
                            )
                            psr = PB.tile([128, TS], f32, tag="pb")
                            for m in range(QLCH):
                                nc.tensor.matmul(
                                    psr,
                                    lhsT=wqr_t[:, m, :],
                                    rhs=ql_sb[:, m, ts(t, TS)],
                                    start=(m == 0),
                                    stop=(m == QLCH - 1),
                                )
                            # rope: qrh = (psr*C + pairswap(psr)*S) * sq
                            tsw = TMPP.tile([128, TS], f32, tag="tmp")
                            nc.vector.stream_shuffle(tsw, psr, SWAP_MASK)
                            t1 = TMPP.tile([128, TS], f32, tag="tmp")
                            nc.vector.tensor_mul(t1, psr, ctab_sb[:, ts(t, TS)])
                            t2 = TMPP.tile([128, TS], f32, tag="tmp")
                            nc.vector.tensor_mul(t2, tsw, stab_sb[:, ts(t, TS)])
                            t3 = TMPP.tile([128, TS], f32, tag="tmp")
                            nc.vector.tensor_add(t3, t1, t2)
                            nc.vector.tensor_mul(
                                qrh_sb[:, ts(t, TS)], t3, sqq_bc[:, ts(t, TS)]
                            )

                        for j in range(NT):
                            n_s = 4 * (j + 1)
                            pctx = PCT.tile([128, TS], f32, tag="pct")
                            eacc = ZR.tile([128, TS], bf, tag="eacc")
                            for s in range(n_s):
                                r = s - 4 * j
                                # diagonal blocks: queries < 128r can't attend
                                off = 128 * r if r > 0 else 0
                                nc2 = TS - off
                                pss = PB.tile([128, TS], f32, tag="pb")
                                nc.tensor.matmul(
                                    pss[:, off:TS],
                                    lhsT=kabs_sb[:, s * 128:(s + 1) * 128],
                                    rhs=qh_sb[:, j * TS + off:(j + 1) * TS],
                                    start=True,
                                    stop=False,
                                )
                                nc.tensor.matmul(
                                    pss[:, off:TS],
                                    lhsT=k_sb[:, s * 128:(s + 1) * 128],
                                    rhs=qrh_sb[:, j * TS + off:(j + 1) * TS],
                                    start=False,
                                    stop=True,
                                )
                                e_t = EP.tile([128, TS], bf, tag="e")
                                nc.scalar.activation(
                                    e_t[:, off:TS], pss[:, off:TS],
                                    AF.Exp, scale=SM_SCALE,
                                )
                                if r >= 0:
                                    nc.vector.tensor_mul(
                                        e_t[:, off:TS], e_t[:, off:TS],
                                        masks_sb[:, 384:384 + nc2],
                                    )
                                if s == 0:
                                    nc.vector.tensor_copy(eacc, e_t)
                                else:
                                    nc.vector.tensor_add(
                                        eacc[:, off:TS], eacc[:, off:TS],
                                        e_t[:, off:TS],
                                    )
                                nc.tensor.matmul(
                                    pctx[:, off:TS],
                                    lhsT=v_sb[:, s, :],
                                    rhs=e_t[:, off:TS],
                                    start=(s == 0),
                                    stop=(s == n_s - 1),
                                )
                            zps = PZB.tile([128, TS], f32, tag="pzb")
                            nc.tensor.matmul(
                                zps, lhsT=ones_mat, rhs=eacc, start=True, stop=True
                            )
                            zfull = TMPP.tile([128, TS], f32, tag="tmp")
                            nc.vector.reciprocal(zfull, zps)
                            nc.vector.tensor_mul(ctx_sb[h][j], pctx, zfull)

                # ---------------- phase 4: output projection -----------------
                with (
                    tc.tile_pool(name="P4", bufs=3) as P4,
                    tc.tile_pool(name="PP4", bufs=3, space="PSUM") as PP4,
                ):
                    for e in range(DCH):
                        o_t = P4.tile([128, T], bf, tag="ot")
                        for t in range(NT):
                            ps = PP4.tile([128, TS], f32, tag="pp4")
                            for q in range(HPG):
                                nc.tensor.matmul(
                                    ps,
                                    lhsT=wout_sb[:, q, e * 128:(e + 1) * 128],
                                    rhs=ctx_sb[q][t],
                                    start=(q == 0),
                                    stop=(q == HPG - 1),
                                )
                            nc.vector.tensor_copy(o_t[:, ts(t, TS)], ps)
                        nc.gpsimd.dma_start(
                            out=outT.ap()[e * 128:(e + 1) * 128, :], in_=o_t
                        )

    nc.compile()
    return nc


def _get_program():
    if "nc" not in _CACHE:
        _CACHE["nc"] = _build_program()
    return _CACHE["nc"]


def _host_prep(inputs):
    """Fold weights on the host and build the 8 per-core input maps."""
    x = np.asarray(inputs["x"], np.float32)
    Wdq = np.asarray(inputs["Wdq"], np.float32)
    qw = np.asarray(inputs["q_norm_w"], np.float32)
    Wuq = np.asarray(inputs["Wuq"], np.float32) * qw[None, :]
    Wqr = np.asarray(inputs["Wqr"], np.float32) * qw[None, :]
    Wdkv = np.asarray(inputs["Wdkv"], np.float32)
    kvw = np.asarray(inputs["kv_norm_w"], np.float32)
    Wuk = np.asarray(inputs["Wuk"], np.float32) * kvw[None, :]
    Wuv = np.asarray(inputs["Wuv"], np.float32) * kvw[None, :]
    Wkr = np.asarray(inputs["Wkr"], np.float32)
    Wout = np.asarray(inputs["Wout"], np.float32)

    inv = 1.0 / (10000.0 ** (np.arange(0, HD, 2, dtype=np.float32) / HD))
    f = np.arange(T, dtype=np.float32)[None, :] * inv[:, None]   # [64, T]
    cosT, sinT = np.cos(f), np.sin(f)
    Ctab = np.repeat(cosT, 2, axis=0)                            # [128, T]
    Stab = np.repeat(sinT, 2, axis=0)
    Stab[0::2, :] *= -1.0                                        # pair-swap sign

    fH = np.arange(KVH, dtype=np.float32)[None, :] * inv[:, None]  # [64, KVH]
    cosH, sinH = np.cos(fH), np.sin(fH)

    def bft(a):
        return np.ascontiguousarray(a).astype(BF16)

    wdqT = bft(Wdq.T)
    wdkvT = bft(Wdkv.T)
    wukT = bft(Wuk.T)
    ctab_b = bft(Ctab)
    stab_b = bft(Stab)

    in_maps = []
    for b in range(B):
        x_T = bft(x[b].T)
        for g in range(G):
            # fold K-rope (fixed rotation per kv-head index) into Wkr
            Wkr_g = Wkr[g * HD:(g + 1) * HD, :]
            we, wo = Wkr_g[0::2, :], Wkr_g[1::2, :]
            c_g, s_g = cosH[:, g][:, None], sinH[:, g][:, None]
            Wkr_eff = np.empty_like(Wkr_g)
            Wkr_eff[0::2, :] = we * c_g - wo * s_g
            Wkr_eff[1::2, :] = we * s_g + wo * c_g

            in_maps.append(
                dict(
                    x_T=x_T,
                    wdqT=wdqT,
                    wdkvT=wdkvT,
                    wkrT=bft(Wkr_eff.T),
                    wuqT=bft(Wuq[g * HPG * HD:(g + 1) * HPG * HD].T),
                    wqrT=bft(Wqr[g * HPG * HD:(g + 1) * HPG * HD].T),
                    wukT=wukT,
                    wuvT=bft(Wuv[g * HD:(g + 1) * HD].T),
                    woutT=bft(Wout[:, g * HPG * HD:(g + 1) * HPG * HD].T),
                    ctab=ctab_b,
                    stab=stab_b,
                )
            )
    return in_maps


def kernel(**inputs):
    global LAST_RESULTS
    from concourse import bass_utils

    nc = _get_program()
    in_maps = _host_prep(inputs)
    res = bass_utils.run_bass_kernel_spmd(
        nc, in_maps, core_ids=list(range(NCORES))
    )
    LAST_RESULTS = res
    out = np.zeros((B, T, D), np.float32)
    for i, r in enumerate(res.results):
        out[i // G] += r["out_T"].T.astype(np.float32)
    return out

